# revision 10
# baseline (speedup 1.0000x reference)
"""Transformer block (LN -> causal MHA -> LN -> MLP, residuals) on 8 trn2 NeuronCores.

Data-parallel over batch: each core runs one [T, C] sequence independently
(no collectives). Matmul inputs are bf16 with fp32 PSUM accumulation;
layernorm, softmax and residuals stay fp32.

Host-side preprocessing folds the layernorm affine params into the adjacent
matmul weights, and folds the V bias through Wo, so the device kernel only
handles plain normalized activations.
"""

import math
import sys

for _p in ("/opt/trn_rl_repo", "/root/.axon_site/_ro/trn_rl_repo"):
    if _p not in sys.path:
        sys.path.append(_p)

import numpy as np
import ml_dtypes

import concourse.bass as bass
import concourse.mybir as mybir
import concourse.tile as tile
from concourse import bacc
from concourse.bass_utils import run_bass_kernel_spmd

B, T, C, H = 8, 1024, 1024, 16
D = C // H
NT = T // 128          # token tiles
NCK = C // 128         # contraction chunks over C
F32 = mybir.dt.float32
BF16 = mybir.dt.bfloat16
AF = mybir.ActivationFunctionType
N_CORES = 8


def _pieces(lo, hi, bound=512):
    """Split [lo, hi) at multiples of `bound` (PSUM bank boundaries)."""
    out = []
    a = lo
    while a < hi:
        b = min(hi, (a // bound + 1) * bound)
        out.append((a, b))
        a = b
    return out


def _layernorm_to_hT(nc, pools, src, hT, ident_sb, eps_sb):
    """src: [128, NT, C] f32 token-major. Writes hT [128, NCK, T] bf16 feature-major."""
    stat_pool, htok_pool, psT = pools
    for i in range(NT):
        xt = src[:, i, :]
        stats = stat_pool.tile([128, 2, 6], F32, tag="lnstats", name="lnstats")
        nc.vector.bn_stats(stats[:, 0, :], xt[:, 0:512])
        nc.vector.bn_stats(stats[:, 1, :], xt[:, 512:1024])
        mv = stat_pool.tile([128, 2], F32, tag="lnmv", name="lnmv")
        nc.vector.bn_aggr(mv, stats)
        std = stat_pool.tile([128, 1], F32, tag="lnstd", name="lnstd")
        nc.scalar.activation(std, mv[:, 1:2], AF.Sqrt, bias=eps_sb, scale=1.0)
        rstd = stat_pool.tile([128, 1], F32, tag="lnrstd", name="lnrstd")
        nc.vector.reciprocal(rstd, std)
        ht = htok_pool.tile([128, C], BF16, tag="htok", name="htok")
        nc.vector.tensor_scalar(
            out=ht, in0=xt, scalar1=mv[:, 0:1], scalar2=rstd,
            op0=mybir.AluOpType.subtract, op1=mybir.AluOpType.mult,
        )
        for cp in range(NCK // 2):
            ps = psT.tile([128, 2, 128], BF16, tag="pst", name="pst")
            nc.tensor.transpose(ps[:, 0, :], ht[:, (2 * cp) * 128:(2 * cp + 1) * 128], ident_sb)
            nc.tensor.transpose(ps[:, 1, :], ht[:, (2 * cp + 1) * 128:(2 * cp + 2) * 128], ident_sb)
            nc.vector.tensor_copy(hT[:, 2 * cp:2 * cp + 2, i * 128:(i + 1) * 128], ps)


def _build_body(tc, io):
    nc = tc.nc
    x_d, wqk_d, wv_d, bqk_d, wo_d, bo_d, wfc_d, bfc_d, wp_d, bp_d, ident_d, maskt_d, out_d = io

    x_v = x_d.rearrange("(n p) c -> p n c", p=128)
    out_v = out_d.rearrange("(n p) c -> p n c", p=128)

    import contextlib
    est = contextlib.ExitStack()
    with est:
        const = est.enter_context(tc.tile_pool(name="const", bufs=1))
        ident_sb = const.tile([128, 128], BF16, tag="ident", name="ident_sb")
        nc.sync.dma_start(ident_sb, ident_d)
        maskt_sb = const.tile([128, 128], BF16, tag="maskt", name="maskt_sb")
        nc.sync.dma_start(maskt_sb, maskt_d)
        ones_sb = const.tile([1, 128], BF16, tag="ones", name="ones_sb")
        nc.vector.memset(ones_sb, 1.0)
        onesf_sb = const.tile([1, 64], F32, tag="onesf", name="onesf_sb")
        nc.vector.memset(onesf_sb, 1.0)
        eps_sb = const.tile([128, 1], F32, tag="eps", name="eps_sb")
        nc.vector.memset(eps_sb, 1e-5)
        bqk_sb = const.tile([128, 16], F32, tag="bqk", name="bqk_sb")
        nc.sync.dma_start(bqk_sb, bqk_d.rearrange("(n p) -> p n", p=128))
        bfc_sb = const.tile([128, 32], F32, tag="bfc", name="bfc_sb")
        nc.sync.dma_start(bfc_sb, bfc_d.rearrange("(n p) -> p n", p=128))
        bp_sb = const.tile([128, 8], F32, tag="bp", name="bp_sb")
        nc.sync.dma_start(bp_sb, bp_d.rearrange("(n p) -> p n", p=128))
        bo_sb = const.tile([1, C], BF16, tag="bo", name="bo_sb")
        nc.sync.dma_start(bo_sb, bo_d.rearrange("(a n) -> a n", a=1))

        ln_small = est.enter_context(tc.tile_pool(name="lnsmall", bufs=3))
        x_pool = est.enter_context(tc.tile_pool(name="xp", bufs=1))
        x_sb = x_pool.tile([128, NT, C], F32, tag="x", name="x_sb")
        yT_pool = est.enter_context(tc.tile_pool(name="ytp", bufs=1))
        yT = yT_pool.tile([128, NCK, T], BF16, tag="yT", name="yT")
        est_attn = est.enter_context(contextlib.ExitStack())
        attn_pool = est_attn.enter_context(tc.tile_pool(name="attnp", bufs=1))
        qkT = attn_pool.tile([128, 2 * NCK, T], BF16, tag="qkT", name="qkT")
        v_sb = attn_pool.tile([128, NT, H, D + 1], BF16, tag="v", name="v_sb")
        wv_sb = attn_pool.tile([128, NCK, C], BF16, tag="wv", name="wv_sb")
        nc.sync.dma_start(wv_sb, wv_d.rearrange("(n p) m -> p n m", p=128))
        nc.vector.memset(v_sb[:, :, :, D:D + 1], 1.0)

        # ---------------- phase 1: load x, LN1, transpose h ----------------
        with tc.tile_pool(name="hTp", bufs=1) as hT_pool, \
             tc.tile_pool(name="psT1", bufs=2, space="PSUM") as psT1, \
             tc.tile_pool(name="psA1", bufs=4, space="PSUM") as psA1, \
             tc.tile_pool(name="wq1", bufs=16) as wq_pool:
            hT = hT_pool.tile([128, NCK, T], BF16, tag="hT", name="hT")
            for i in range(NT):
                nc.sync.dma_start(x_sb[:, i, :], x_v[:, i, :])
            _layernorm_to_hT(nc, (ln_small, ln_small, psT1), x_sb, hT, ident_sb, eps_sb)

            # ---------------- phase 2: qkv projections ----------------
            # q,k feature-major: qkT[f, t] = sum_c Wqk[c, f] * hT[c, t]  (+bias via ACT)
            for fg in range(8):  # 256-wide feature groups over 2C
                wts = []
                for ck in range(NCK):
                    wt = wq_pool.tile([128, 256], BF16, tag="wqk", name="wqkt")
                    nc.sync.dma_start(wt, wqk_d[ck * 128:(ck + 1) * 128, fg * 256:(fg + 1) * 256])
                    wts.append(wt)
                for fl in range(2):
                    fn = fg * 2 + fl
                    for tsp in range(2):
                        ps = psA1.tile([128, 512], F32, tag="psqkv", name="psqkv")
                        for ck in range(NCK):
                            nc.tensor.matmul(
                                ps, lhsT=wts[ck][:, fl * 128:(fl + 1) * 128],
                                rhs=hT[:, ck, tsp * 512:(tsp + 1) * 512],
                                start=(ck == 0), stop=(ck == NCK - 1),
                            )
                        nc.scalar.activation(
                            qkT[:, fn, tsp * 512:(tsp + 1) * 512], ps,
                            AF.Identity, bias=bqk_sb[:, fn:fn + 1], scale=1.0,
                        )
            # v token-major: v[t, n] = sum_c hT[c, t] * Wv[c, n]  (bias folded into bo)
            for ti in range(NT):
                for nsp in range(2):
                    ps = psA1.tile([128, 512], F32, tag="psqkv", name="psqkv")
                    for ck in range(NCK):
                        nc.tensor.matmul(
                            ps, lhsT=hT[:, ck, ti * 128:(ti + 1) * 128],
                            rhs=wv_sb[:, ck, nsp * 512:(nsp + 1) * 512],
                            start=(ck == 0), stop=(ck == NCK - 1),
                        )
                    nc.vector.tensor_copy(
                        v_sb[:, ti, nsp * 8:(nsp + 1) * 8, 0:D],
                        ps.rearrange("p (h d) -> p h d", h=8),
                    )

        # ---------------- phase 3: attention (per head) ----------------
        with tc.tile_pool(name="ptp", bufs=2) as pt_pool, \
             tc.tile_pool(name="asml", bufs=2) as asml, \
             tc.tile_pool(name="psS", bufs=2, space="PSUM") as psS, \
             tc.tile_pool(name="psY", bufs=1, space="PSUM") as psY:
            inv_sqrt_c = 1.0 / math.sqrt(C)
            for h in range(H):
                po = 64 * (h % 2)
                hc = h // 2
                qT = qkT[po:po + 64, hc, :]
                kT = qkT[po:po + 64, NCK + hc, :]
                PT = pt_pool.tile([128, NT, T], BF16, tag="pt", name="PT")
                for j in range(NT):
                    lo = j * 128
                    ss = psS.tile([128, T], F32, tag="st", name="ss")
                    for (a, b) in _pieces(lo, T):
                        nc.tensor.matmul(
                            ss[:, a:b], lhsT=kT[:, lo:lo + 128], rhs=qT[:, a:b],
                            start=True, stop=True,
                        )
                    nc.scalar.activation(PT[:, j, lo:T], ss[:, lo:T], AF.Exp, scale=inv_sqrt_c)
                    nc.vector.tensor_mul(PT[:, j, lo:lo + 128], PT[:, j, lo:lo + 128], maskt_sb)
                yps = psY.tile([65, T], F32, tag="y", name="yps")
                for j in range(NT):
                    lv = v_sb[:, j, h, :]
                    for (a, b) in _pieces(j * 128, T):
                        last = (j == min(NT - 1, (b - 1) // 128))
                        nc.tensor.matmul(
                            yps[:, a:b], lhsT=lv, rhs=PT[:, j, a:b],
                            start=(j == 0), stop=last, skip_group_check=True,
                        )
                recip = asml.tile([1, T], F32, tag="recip", name="recip")
                nc.vector.reciprocal(recip, yps[64:65, :])
                rb = psS.tile([64, T], F32, tag="rb", bufs=1, name="rb")
                for (a, b) in ((0, 512), (512, 1024)):
                    nc.tensor.matmul(rb[:, a:b], lhsT=onesf_sb[0:1, 0:64], rhs=recip[0:1, a:b],
                                     start=True, stop=True)
                rbs = asml.tile([64, T], F32, tag="rbs", name="rbs")
                nc.scalar.copy(rbs, rb)
                if po == 0:
                    nc.vector.tensor_mul(yT[0:64, hc, :], yps[0:64, :], rbs)
                else:
                    ytmp = asml.tile([64, T], BF16, tag="ytmp", name="ytmp")
                    nc.vector.tensor_mul(ytmp, yps[0:64, :], rbs)
                    nc.sync.dma_start(yT[64:128, hc, :], ytmp)

        est_attn.close()  # free qkT/v/wv space
        x2 = x_sb  # attention residual is written in place

        # ---------------- phase 4: attention out-proj + residual ----------------
        with tc.tile_pool(name="wop", bufs=1) as wo_pool, \
             tc.tile_pool(name="psA2", bufs=4, space="PSUM") as psA2:
            wo_sb = wo_pool.tile([128, NCK, C], BF16, tag="wo", name="wo_sb")
            nc.sync.dma_start(wo_sb, wo_d.rearrange("(n p) m -> p n m", p=128))
            for ti in range(NT):
                for nsp in range(2):
                    ps = psA2.tile([128, 512], F32, tag="pswo", name="pswo")
                    for cn in range(NCK):
                        nc.tensor.matmul(
                            ps, lhsT=yT[:, cn, ti * 128:(ti + 1) * 128],
                            rhs=wo_sb[:, cn, nsp * 512:(nsp + 1) * 512],
                            start=(cn == 0), stop=False,
                        )
                    nc.tensor.matmul(ps, lhsT=ones_sb[0:1, 0:128],
                                     rhs=bo_sb[0:1, nsp * 512:(nsp + 1) * 512],
                                     start=False, stop=True)
                    nc.vector.tensor_add(
                        x2[:, ti, nsp * 512:(nsp + 1) * 512], ps,
                        x_sb[:, ti, nsp * 512:(nsp + 1) * 512],
                    )

        # ---------------- phase 5/6: LN2 + FC(gelu) ----------------
        mlp_pool = est.enter_context(tc.tile_pool(name="mlpp", bufs=1))
        mT = mlp_pool.tile([128, 4 * NCK, T], BF16, tag="mT", name="mT")
        outT = mlp_pool.tile([128, NCK, T], BF16, tag="outT", name="outT")
        with tc.tile_pool(name="h2Tp", bufs=1) as h2T_pool, \
             tc.tile_pool(name="psT2", bufs=2, space="PSUM") as psT2, \
             tc.tile_pool(name="psA3", bufs=4, space="PSUM") as psA3, \
             tc.tile_pool(name="wf1", bufs=16) as wf_pool:
            h2T = h2T_pool.tile([128, NCK, T], BF16, tag="h2T", name="h2T")
            _layernorm_to_hT(nc, (ln_small, ln_small, psT2), x2, h2T, ident_sb, eps_sb)
            for fg in range(16):  # 256-wide feature groups over 4C
                wts = []
                for ck in range(NCK):
                    wt = wf_pool.tile([128, 256], BF16, tag="wfc", name="wfct")
                    nc.sync.dma_start(wt, wfc_d[ck * 128:(ck + 1) * 128, fg * 256:(fg + 1) * 256])
                    wts.append(wt)
                for fl in range(2):
                    fn = fg * 2 + fl
                    for tsp in range(2):
                        ps = psA3.tile([128, 512], F32, tag="psfc", name="psfc")
                        for ck in range(NCK):
                            nc.tensor.matmul(
                                ps, lhsT=wts[ck][:, fl * 128:(fl + 1) * 128],
                                rhs=h2T[:, ck, tsp * 512:(tsp + 1) * 512],
                                start=(ck == 0), stop=(ck == NCK - 1),
                            )
                        nc.scalar.activation(
                            mT[:, fn, tsp * 512:(tsp + 1) * 512], ps,
                            AF.Gelu_apprx_tanh, bias=bfc_sb[:, fn:fn + 1], scale=1.0,
                        )

        # ---------------- phase 7: Wp (feature-major out) ----------------
        with tc.tile_pool(name="wpp", bufs=6) as wp_pool, \
             tc.tile_pool(name="psW", bufs=4, space="PSUM") as psW, \
             tc.tile_pool(name="psT3", bufs=2, space="PSUM") as psT3, \
             tc.tile_pool(name="outp", bufs=2) as out_pool:
            for cg in range(4):  # output feature groups of 256
                pss = [[psW.tile([128, 512], F32, tag="pswp", name="pswp")
                        for _ in range(2)] for _ in range(2)]
                for kn in range(4 * NCK):
                    wt = wp_pool.tile([128, 256], BF16, tag="wp", name="wpt")
                    nc.sync.dma_start(wt, wp_d[kn * 128:(kn + 1) * 128, cg * 256:(cg + 1) * 256])
                    for cl in range(2):
                        for tsp in range(2):
                            nc.tensor.matmul(
                                pss[cl][tsp], lhsT=wt[:, cl * 128:(cl + 1) * 128],
                                rhs=mT[:, kn, tsp * 512:(tsp + 1) * 512],
                                start=(kn == 0), stop=(kn == 4 * NCK - 1),
                            )
                for cl in range(2):
                    cn = cg * 2 + cl
                    for tsp in range(2):
                        nc.scalar.activation(
                            outT[:, cn, tsp * 512:(tsp + 1) * 512], pss[cl][tsp],
                            AF.Identity, bias=bp_sb[:, cn:cn + 1], scale=1.0,
                        )
            # ---------------- phase 8: transpose back + residual + store ----------------
            for ti in range(NT):
                outt = out_pool.tile([128, C], F32, tag="osb", name="outt")
                for cg in range(2):
                    ps4 = psT3.tile([128, 4, 128], BF16, tag="pst3", name="ps4")
                    for cl in range(4):
                        cj = cg * 4 + cl
                        nc.tensor.transpose(ps4[:, cl, :], outT[:, cj, ti * 128:(ti + 1) * 128], ident_sb)
                    nc.vector.tensor_add(
                        outt[:, cg * 512:(cg + 1) * 512].rearrange("p (a b) -> p a b", a=4),
                        ps4,
                        x2[:, ti, cg * 512:(cg + 1) * 512].rearrange("p (a b) -> p a b", a=4),
                    )
                nc.sync.dma_start(out_v[:, ti, :], outt)


def build_module():
    nc = bacc.Bacc("TRN2", target_bir_lowering=False, debug=False)

    def din(name, shape, dtype):
        return nc.dram_tensor(name, list(shape), dtype, kind="ExternalInput").ap()

    io = (
        din("x", (T, C), F32),
        din("wqk", (C, 2 * C), BF16),
        din("wv", (C, C), BF16),
        din("bqk", (2 * C,), F32),
        din("wo", (C, C), BF16),
        din("bo", (C,), BF16),
        din("wfc", (C, 4 * C), BF16),
        din("bfc", (4 * C,), F32),
        din("wp", (4 * C, C), BF16),
        din("bp", (C,), F32),
        din("ident", (128, 128), BF16),
        din("maskt", (128, 128), BF16),
        nc.dram_tensor("out", [T, C], F32, kind="ExternalOutput").ap(),
    )
    with tile.TileContext(nc) as tc:
        _build_body(tc, io)
    nc.compile()
    return nc


def host_prepare(inputs):
    """Fold LN affine params / v-bias into weights; cast matmul weights to bf16."""
    bf = ml_dtypes.bfloat16
    x = np.asarray(inputs["x"], np.float32)
    Wqkv = np.asarray(inputs["Wqkv"], np.float64)
    bqkv = np.asarray(inputs["bqkv"], np.float64)
    Wo = np.asarray(inputs["Wo"], np.float64)
    bo = np.asarray(inputs["bo"], np.float64)
    ln1_w = np.asarray(inputs["ln1_w"], np.float64)
    ln1_b = np.asarray(inputs["ln1_b"], np.float64)
    ln2_w = np.asarray(inputs["ln2_w"], np.float64)
    ln2_b = np.asarray(inputs["ln2_b"], np.float64)
    Wfc = np.asarray(inputs["Wfc"], np.float64)
    bfc = np.asarray(inputs["bfc"], np.float64)
    Wp = np.asarray(inputs["Wp"], np.float64)
    bp = np.asarray(inputs["bp"], np.float64)

    Wqkv_f = ln1_w[:, None] * Wqkv
    bqkv_f = bqkv + ln1_b @ Wqkv
    bo_f = bo + bqkv_f[2 * C:] @ Wo
    Wfc_f = ln2_w[:, None] * Wfc
    bfc_f = bfc + ln2_b @ Wfc

    common = {
        "wqk": Wqkv_f[:, :2 * C].astype(bf),
        "wv": Wqkv_f[:, 2 * C:].astype(bf),
        "bqk": bqkv_f[:2 * C].astype(np.float32),
        "wo": Wo.astype(bf),
        "bo": bo_f.astype(bf),
        "wfc": Wfc_f.astype(bf),
        "bfc": bfc_f.astype(np.float32),
        "wp": Wp.astype(bf),
        "bp": bp.astype(np.float32),
        "ident": np.eye(128, dtype=bf),
        "maskt": np.triu(np.ones((128, 128))).astype(bf),
    }
    return x, common


_NC_CACHE = None


def get_module():
    global _NC_CACHE
    if _NC_CACHE is None:
        _NC_CACHE = build_module()
    return _NC_CACHE


def run_with_results(inputs, **run_kwargs):
    x, common = host_prepare(inputs)
    nc = get_module()
    in_maps = [dict(common, x=np.ascontiguousarray(x[b])) for b in range(B)]
    res = run_bass_kernel_spmd(nc, in_maps, core_ids=list(range(N_CORES)), **run_kwargs)
    out = np.stack([res.results[b]["out"] for b in range(B)]).astype(np.float32)
    return out, res


def kernel(**inputs):
    return run_with_results(inputs)[0]


# revision 29
# speedup vs baseline: 1.2033x; 1.2033x over previous
"""Transformer block (LN -> causal MHA -> LN -> MLP, residuals) on 8 trn2 NeuronCores.

Data-parallel over batch: each core runs one [T, C] sequence independently
(no collectives). Matmul inputs are bf16 with fp32 PSUM accumulation;
layernorm, softmax and residuals stay fp32.

Host-side preprocessing folds the layernorm affine params into the adjacent
matmul weights, and folds the V bias through Wo, so the device kernel only
handles plain normalized activations.
"""

import math
import sys

for _p in ("/opt/trn_rl_repo", "/root/.axon_site/_ro/trn_rl_repo"):
    if _p not in sys.path:
        sys.path.append(_p)

import numpy as np
import ml_dtypes

import concourse.bass as bass
import concourse.mybir as mybir
import concourse.tile as tile
from concourse import bacc
from concourse.bass_utils import run_bass_kernel_spmd

B, T, C, H = 8, 1024, 1024, 16
D = C // H
NT = T // 128          # token tiles
NCK = C // 128         # contraction chunks over C
F32 = mybir.dt.float32
BF16 = mybir.dt.bfloat16
AF = mybir.ActivationFunctionType
N_CORES = 8


def _pieces(lo, hi, bound=512):
    """Split [lo, hi) at multiples of `bound` (PSUM bank boundaries)."""
    out = []
    a = lo
    while a < hi:
        b = min(hi, (a // bound + 1) * bound)
        out.append((a, b))
        a = b
    return out


def _layernorm_to_hT(nc, pools, src, hT, ident_sb, eps_sb):
    """src: [128, NT, C] f32 token-major. Writes hT [128, NCK, T] bf16 feature-major."""
    stat_pool, htok_pool, psT = pools
    for i in range(NT):
        xt = src[:, i, :]
        stats = stat_pool.tile([128, 2, 6], F32, tag="lnstats", name="lnstats")
        nc.vector.bn_stats(stats[:, 0, :], xt[:, 0:512])
        nc.vector.bn_stats(stats[:, 1, :], xt[:, 512:1024])
        mv = stat_pool.tile([128, 2], F32, tag="lnmv", name="lnmv")
        nc.vector.bn_aggr(mv, stats)
        std = stat_pool.tile([128, 1], F32, tag="lnstd", name="lnstd")
        nc.scalar.activation(std, mv[:, 1:2], AF.Sqrt, bias=eps_sb, scale=1.0)
        rstd = stat_pool.tile([128, 1], F32, tag="lnrstd", name="lnrstd")
        nc.vector.reciprocal(rstd, std)
        ht = htok_pool.tile([128, C], BF16, tag="htok", name="htok")
        nc.vector.tensor_scalar(
            out=ht, in0=xt, scalar1=mv[:, 0:1], scalar2=rstd,
            op0=mybir.AluOpType.subtract, op1=mybir.AluOpType.mult,
        )
        for cp in range(NCK // 2):
            ps = psT.tile([128, 2, 128], BF16, tag="pst", name="pst")
            nc.tensor.transpose(ps[:, 0, :], ht[:, (2 * cp) * 128:(2 * cp + 1) * 128], ident_sb)
            nc.tensor.transpose(ps[:, 1, :], ht[:, (2 * cp + 1) * 128:(2 * cp + 2) * 128], ident_sb)
            nc.vector.tensor_copy(hT[:, 2 * cp:2 * cp + 2, i * 128:(i + 1) * 128], ps)


def _build_body(tc, io, taps=None):
    nc = tc.nc
    x_d, wqk_d, wv_d, bqk_d, wo_d, bo_d, wfc_d, bfc_d, wp_d, bp_d, ident_d, maskt_d, out_d = io

    x_v = x_d.rearrange("(n p) c -> p n c", p=128)
    out_v = out_d.rearrange("(n p) c -> p n c", p=128)

    import contextlib
    est = contextlib.ExitStack()
    with est:
        const = est.enter_context(tc.tile_pool(name="const", bufs=1))
        ident_sb = const.tile([128, 128], BF16, tag="ident", name="ident_sb")
        nc.sync.dma_start(ident_sb, ident_d)
        maskt_sb = const.tile([128, 128], BF16, tag="maskt", name="maskt_sb")
        nc.sync.dma_start(maskt_sb, maskt_d)
        ones_sb = const.tile([1, 128], BF16, tag="ones", name="ones_sb")
        nc.vector.memset(ones_sb, 1.0)
        onesf_sb = const.tile([1, 64], F32, tag="onesf", name="onesf_sb")
        nc.vector.memset(onesf_sb, 1.0)
        eps_sb = const.tile([128, 1], F32, tag="eps", name="eps_sb")
        nc.vector.memset(eps_sb, 1e-5)
        bqk_sb = const.tile([128, 16], F32, tag="bqk", name="bqk_sb")
        nc.sync.dma_start(bqk_sb, bqk_d.rearrange("(n p) -> p n", p=128))
        bfc_sb = const.tile([128, 32], F32, tag="bfc", name="bfc_sb")
        nc.sync.dma_start(bfc_sb, bfc_d.rearrange("(n p) -> p n", p=128))
        bp_sb = const.tile([128, 8], F32, tag="bp", name="bp_sb")
        nc.sync.dma_start(bp_sb, bp_d.rearrange("(n p) -> p n", p=128))
        bo_sb = const.tile([1, C], BF16, tag="bo", name="bo_sb")
        nc.sync.dma_start(bo_sb, bo_d.rearrange("(a n) -> a n", a=1))

        ln_small = est.enter_context(tc.tile_pool(name="lnsmall", bufs=3))
        x_pool = est.enter_context(tc.tile_pool(name="xp", bufs=1))
        x_sb = x_pool.tile([128, NT, C], F32, tag="x", name="x_sb")
        yT_pool = est.enter_context(tc.tile_pool(name="ytp", bufs=1))
        yT = yT_pool.tile([128, NCK, T], BF16, tag="yT", name="yT")
        est_attn = est.enter_context(contextlib.ExitStack())
        attn_pool = est_attn.enter_context(tc.tile_pool(name="attnp", bufs=1))
        qkT = attn_pool.tile([128, 2 * NCK, T], BF16, tag="qkT", name="qkT")
        v_sb = attn_pool.tile([128, NT, H, D + 1], BF16, tag="v", name="v_sb")
        wv_sb = attn_pool.tile([128, NCK, C], BF16, tag="wv", name="wv_sb")
        nc.sync.dma_start(wv_sb, wv_d.rearrange("(n p) m -> p n m", p=128))
        nc.vector.memset(v_sb[:, :, :, D:D + 1], 1.0)

        # ---------------- phase 1: load x, LN1, transpose h ----------------
        with tc.tile_pool(name="hTp", bufs=1) as hT_pool, \
             tc.tile_pool(name="psT1", bufs=2, space="PSUM") as psT1, \
             tc.tile_pool(name="psA1", bufs=4, space="PSUM") as psA1, \
             tc.tile_pool(name="wq1", bufs=16) as wq_pool:
            hT = hT_pool.tile([128, NCK, T], BF16, tag="hT", name="hT")
            for i in range(NT):
                nc.sync.dma_start(x_sb[:, i, :], x_v[:, i, :])
            _layernorm_to_hT(nc, (ln_small, ln_small, psT1), x_sb, hT, ident_sb, eps_sb)

            # ---------------- phase 2: qkv projections ----------------
            # v token-major first (only needs per-token-tile hT, so PE warms up
            # while the qk feature groups' weights stream in)
            for ti in range(NT):
                for nsp in range(2):
                    ps = psA1.tile([128, 512], F32, tag="psqkv", name="psqkv")
                    for ck in range(NCK):
                        nc.tensor.matmul(
                            ps, lhsT=hT[:, ck, ti * 128:(ti + 1) * 128],
                            rhs=wv_sb[:, ck, nsp * 512:(nsp + 1) * 512],
                            start=(ck == 0), stop=(ck == NCK - 1),
                        )
                    nc.vector.tensor_copy(
                        v_sb[:, ti, nsp * 8:(nsp + 1) * 8, 0:D],
                        ps.rearrange("p (h d) -> p h d", h=8),
                    )
            # q,k feature-major: qkT[f, t] = sum_c Wqk[c, f] * hT[c, t]  (+bias via ACT)
            # Feature groups ordered so q-chunk / k-chunk pairs of the low heads
            # land first (heads can start scoring before all of qk is done).
            for fg in (0, 4, 1, 5, 2, 6, 3, 7):  # 256-wide feature groups over 2C
                wts = []
                for ck in range(NCK):
                    wt = wq_pool.tile([128, 256], BF16, tag="wqk", name="wqkt")
                    nc.sync.dma_start(wt, wqk_d[ck * 128:(ck + 1) * 128, fg * 256:(fg + 1) * 256])
                    wts.append(wt)
                for fl in range(2):
                    fn = fg * 2 + fl
                    for tsp in range(2):
                        ps = psA1.tile([128, 512], F32, tag="psqkv", name="psqkv")
                        for ck in range(NCK):
                            nc.tensor.matmul(
                                ps, lhsT=wts[ck][:, fl * 128:(fl + 1) * 128],
                                rhs=hT[:, ck, tsp * 512:(tsp + 1) * 512],
                                start=(ck == 0), stop=(ck == NCK - 1),
                            )
                        nc.scalar.activation(
                            qkT[:, fn, tsp * 512:(tsp + 1) * 512], ps,
                            AF.Identity, bias=bqk_sb[:, fn:fn + 1], scale=1.0,
                        )

        # ---------------- phase 3: attention (per head) ----------------
        # Heads are software-pipelined (scores for head h interleave with PV of
        # head h-1 so the in-order PE never stalls on the exp/mask chain).
        # y is stored UNNORMALIZED; rowsums (the ones-column of v_aug) are
        # staged into [16, T], reciprocal'd once, broadcast via GpSimd, and the
        # normalization happens as 8 head-pair muls at the end.
        with tc.tile_pool(name="ptp", bufs=2) as pt_pool, \
             tc.tile_pool(name="asml", bufs=2) as asml, \
             tc.tile_pool(name="psS", bufs=2, space="PSUM") as psS, \
             tc.tile_pool(name="psY", bufs=2, space="PSUM") as psY:
            inv_sqrt_c = 1.0 / math.sqrt(C)

            def scores_phase(h):
                po = 64 * (h % 2)
                hc = h // 2
                qT = qkT[po:po + 64, hc, :]
                kT = qkT[po:po + 64, NCK + hc, :]
                PT = pt_pool.tile([128, NT, T], BF16, tag="pt", name="PT")
                for j in range(NT):
                    lo = j * 128
                    ss = psS.tile([128, T], F32, tag="st", name="ss")
                    for (a, b) in _pieces(lo, T):
                        nc.tensor.matmul(
                            ss[:, a:b], lhsT=kT[:, lo:lo + 128], rhs=qT[:, a:b],
                            start=True, stop=True,
                        )
                    nc.scalar.activation(PT[:, j, lo:T], ss[:, lo:T], AF.Exp, scale=inv_sqrt_c)
                    nc.vector.tensor_mul(PT[:, j, lo:lo + 128], PT[:, j, lo:lo + 128], maskt_sb)
                return PT

            def pv_phase(h, PT):
                po = 64 * (h % 2)
                hc = h // 2
                yps = psY.tile([65, T], F32, tag="y", name="yps")
                for j in range(NT):
                    lv = v_sb[:, j, h, :]
                    for (a, b) in _pieces(j * 128, T):
                        last = (j == min(NT - 1, (b - 1) // 128))
                        nc.tensor.matmul(
                            yps[:, a:b], lhsT=lv, rhs=PT[:, j, a:b],
                            start=(j == 0), stop=last, skip_group_check=True,
                        )
                # Normalization: rowsum (PSUM row 64) -> SBUF -> fast approx
                # reciprocal -> bf16 -> PE ones-broadcast to [64, T] -> fused
                # into the PSUM->SBUF drain of y. (The recip is cast to bf16 for
                # the broadcast matmul; y itself is bf16, so this costs nothing
                # extra in accuracy.)
                # The custom-DVE reciprocal mishandles operands at a nonzero
                # base partition, so hop the rowsum to partition 0 with a plain
                # copy first and keep the whole chain at base 0.
                srow = asml.tile([1, 2, T], F32, tag="srow", name="srow")
                nc.vector.tensor_copy(srow[:, 0, :], yps[64:65, :])
                nc.vector.reciprocal_approx_fast(srow[:, 1, :], srow[:, 0, :])
                rbf = asml.tile([1, T], BF16, tag="rbf", name="rbf")
                nc.vector.tensor_copy(rbf, srow[:, 1, :])
                rbps = psS.tile([64, T], F32, tag="st", name="rbps")
                for (a, b) in ((0, 512), (512, 1024)):
                    nc.tensor.matmul(rbps[:, a:b], lhsT=ones_sb[0:1, 0:64], rhs=rbf[0:1, a:b],
                                     start=True, stop=True)
                rbs = asml.tile([64, T], F32, tag="rbs", name="rbs")
                nc.scalar.copy(rbs, rbps)
                if taps is not None:
                    nc.sync.dma_start(taps["sums"][h:h + 1, :], srow[:, 0, :])
                    nc.sync.dma_start(taps["recips"][h:h + 1, :], srow[:, 1, :])
                    nc.sync.dma_start(taps["rbsrow"][h:h + 1, :], rbs[0:1, :])
                if po == 0:
                    nc.vector.tensor_mul(yT[0:64, hc, :], yps[0:64, :], rbs)
                else:
                    ytmp = asml.tile([64, T], BF16, tag="ytmp", name="ytmp")
                    nc.vector.tensor_mul(ytmp, yps[0:64, :], rbs)
                    nc.sync.dma_start(yT[64:128, hc, :], ytmp)

            import os as _os
            if _os.environ.get("ATTN_NO_PIPELINE", "0") == "1":
                for h in range(H):
                    pv_phase(h, scores_phase(h))
            else:
                prev = None
                for h in range(H):
                    PT = scores_phase(h)
                    if prev is not None:
                        pv_phase(h - 1, prev)
                    prev = PT
                pv_phase(H - 1, prev)

        est_attn.close()  # free qkT/v/wv space
        x2 = x_sb  # attention residual is written in place

        if taps is not None:
            nc.sync.dma_start(taps["yT"], yT)

        # ---------------- phase 4: attention out-proj + residual ----------------
        with tc.tile_pool(name="wop", bufs=1) as wo_pool, \
             tc.tile_pool(name="psA2", bufs=4, space="PSUM") as psA2:
            wo_sb = wo_pool.tile([128, NCK, C], BF16, tag="wo", name="wo_sb")
            nc.sync.dma_start(wo_sb, wo_d.rearrange("(n p) m -> p n m", p=128))
            for ti in range(NT):
                for nsp in range(2):
                    ps = psA2.tile([128, 512], F32, tag="pswo", name="pswo")
                    for cn in range(NCK):
                        nc.tensor.matmul(
                            ps, lhsT=yT[:, cn, ti * 128:(ti + 1) * 128],
                            rhs=wo_sb[:, cn, nsp * 512:(nsp + 1) * 512],
                            start=(cn == 0), stop=False,
                        )
                    nc.tensor.matmul(ps, lhsT=ones_sb[0:1, 0:128],
                                     rhs=bo_sb[0:1, nsp * 512:(nsp + 1) * 512],
                                     start=False, stop=True)
                    nc.vector.tensor_add(
                        x2[:, ti, nsp * 512:(nsp + 1) * 512], ps,
                        x_sb[:, ti, nsp * 512:(nsp + 1) * 512],
                    )

        # ---------------- phase 5/6: LN2 + FC(gelu) ----------------
        mlp_pool = est.enter_context(tc.tile_pool(name="mlpp", bufs=1))
        mT = mlp_pool.tile([128, 4 * NCK, T], BF16, tag="mT", name="mT")
        outT = mlp_pool.tile([128, NCK, T], BF16, tag="outT", name="outT")
        with tc.tile_pool(name="h2Tp", bufs=1) as h2T_pool, \
             tc.tile_pool(name="psT2", bufs=2, space="PSUM") as psT2, \
             tc.tile_pool(name="psA3", bufs=4, space="PSUM") as psA3, \
             tc.tile_pool(name="wf1", bufs=16) as wf_pool:
            h2T = h2T_pool.tile([128, NCK, T], BF16, tag="h2T", name="h2T")
            _layernorm_to_hT(nc, (ln_small, ln_small, psT2), x2, h2T, ident_sb, eps_sb)
            for fg in range(16):  # 256-wide feature groups over 4C
                wts = []
                for ck in range(NCK):
                    wt = wf_pool.tile([128, 256], BF16, tag="wfc", name="wfct")
                    nc.sync.dma_start(wt, wfc_d[ck * 128:(ck + 1) * 128, fg * 256:(fg + 1) * 256])
                    wts.append(wt)
                for fl in range(2):
                    fn = fg * 2 + fl
                    for tsp in range(2):
                        ps = psA3.tile([128, 512], F32, tag="psfc", name="psfc")
                        for ck in range(NCK):
                            nc.tensor.matmul(
                                ps, lhsT=wts[ck][:, fl * 128:(fl + 1) * 128],
                                rhs=h2T[:, ck, tsp * 512:(tsp + 1) * 512],
                                start=(ck == 0), stop=(ck == NCK - 1),
                            )
                        nc.scalar.activation(
                            mT[:, fn, tsp * 512:(tsp + 1) * 512], ps,
                            AF.Gelu_apprx_tanh, bias=bfc_sb[:, fn:fn + 1], scale=1.0,
                        )

        # ---------------- phase 7: Wp (feature-major out) ----------------
        with tc.tile_pool(name="wpp", bufs=6) as wp_pool, \
             tc.tile_pool(name="psW", bufs=4, space="PSUM") as psW, \
             tc.tile_pool(name="psT3", bufs=2, space="PSUM") as psT3, \
             tc.tile_pool(name="outp", bufs=2) as out_pool:
            for cg in range(4):  # output feature groups of 256
                pss = [[psW.tile([128, 512], F32, tag="pswp", name="pswp")
                        for _ in range(2)] for _ in range(2)]
                for kn in range(4 * NCK):
                    wt = wp_pool.tile([128, 256], BF16, tag="wp", name="wpt")
                    nc.sync.dma_start(wt, wp_d[kn * 128:(kn + 1) * 128, cg * 256:(cg + 1) * 256])
                    for cl in range(2):
                        for tsp in range(2):
                            nc.tensor.matmul(
                                pss[cl][tsp], lhsT=wt[:, cl * 128:(cl + 1) * 128],
                                rhs=mT[:, kn, tsp * 512:(tsp + 1) * 512],
                                start=(kn == 0), stop=(kn == 4 * NCK - 1),
                            )
                for cl in range(2):
                    cn = cg * 2 + cl
                    for tsp in range(2):
                        nc.scalar.activation(
                            outT[:, cn, tsp * 512:(tsp + 1) * 512], pss[cl][tsp],
                            AF.Identity, bias=bp_sb[:, cn:cn + 1], scale=1.0,
                        )
            # ---------------- phase 8: transpose back + residual + store ----------------
            for ti in range(NT):
                outt = out_pool.tile([128, C], F32, tag="osb", name="outt")
                for cg in range(2):
                    ps4 = psT3.tile([128, 4, 128], BF16, tag="pst3", name="ps4")
                    for cl in range(4):
                        cj = cg * 4 + cl
                        nc.tensor.transpose(ps4[:, cl, :], outT[:, cj, ti * 128:(ti + 1) * 128], ident_sb)
                    nc.vector.tensor_add(
                        outt[:, cg * 512:(cg + 1) * 512].rearrange("p (a b) -> p a b", a=4),
                        ps4,
                        x2[:, ti, cg * 512:(cg + 1) * 512].rearrange("p (a b) -> p a b", a=4),
                    )
                nc.sync.dma_start(out_v[:, ti, :], outt)


def build_module(debug_taps=False):
    nc = bacc.Bacc("TRN2", target_bir_lowering=False, debug=False)

    def din(name, shape, dtype):
        return nc.dram_tensor(name, list(shape), dtype, kind="ExternalInput").ap()

    taps = None
    if debug_taps:
        taps = {
            "sums": nc.dram_tensor("dbg_sums", [H, T], F32, kind="ExternalOutput").ap(),
            "recips": nc.dram_tensor("dbg_recips", [H, T], F32, kind="ExternalOutput").ap(),
            "rbsrow": nc.dram_tensor("dbg_rbsrow", [H, T], F32, kind="ExternalOutput").ap(),
            "yT": nc.dram_tensor("dbg_yT", [128, NCK, T], BF16, kind="ExternalOutput").ap(),
        }

    io = (
        din("x", (T, C), F32),
        din("wqk", (C, 2 * C), BF16),
        din("wv", (C, C), BF16),
        din("bqk", (2 * C,), F32),
        din("wo", (C, C), BF16),
        din("bo", (C,), BF16),
        din("wfc", (C, 4 * C), BF16),
        din("bfc", (4 * C,), F32),
        din("wp", (4 * C, C), BF16),
        din("bp", (C,), F32),
        din("ident", (128, 128), BF16),
        din("maskt", (128, 128), BF16),
        nc.dram_tensor("out", [T, C], F32, kind="ExternalOutput").ap(),
    )
    with tile.TileContext(nc) as tc:
        _build_body(tc, io, taps=taps)
    nc.compile()
    return nc


def host_prepare(inputs):
    """Fold LN affine params / v-bias into weights; cast matmul weights to bf16."""
    bf = ml_dtypes.bfloat16
    x = np.asarray(inputs["x"], np.float32)
    Wqkv = np.asarray(inputs["Wqkv"], np.float64)
    bqkv = np.asarray(inputs["bqkv"], np.float64)
    Wo = np.asarray(inputs["Wo"], np.float64)
    bo = np.asarray(inputs["bo"], np.float64)
    ln1_w = np.asarray(inputs["ln1_w"], np.float64)
    ln1_b = np.asarray(inputs["ln1_b"], np.float64)
    ln2_w = np.asarray(inputs["ln2_w"], np.float64)
    ln2_b = np.asarray(inputs["ln2_b"], np.float64)
    Wfc = np.asarray(inputs["Wfc"], np.float64)
    bfc = np.asarray(inputs["bfc"], np.float64)
    Wp = np.asarray(inputs["Wp"], np.float64)
    bp = np.asarray(inputs["bp"], np.float64)

    Wqkv_f = ln1_w[:, None] * Wqkv
    bqkv_f = bqkv + ln1_b @ Wqkv
    bo_f = bo + bqkv_f[2 * C:] @ Wo
    Wfc_f = ln2_w[:, None] * Wfc
    bfc_f = bfc + ln2_b @ Wfc

    common = {
        "wqk": Wqkv_f[:, :2 * C].astype(bf),
        "wv": Wqkv_f[:, 2 * C:].astype(bf),
        "bqk": bqkv_f[:2 * C].astype(np.float32),
        "wo": Wo.astype(bf),
        "bo": bo_f.astype(bf),
        "wfc": Wfc_f.astype(bf),
        "bfc": bfc_f.astype(np.float32),
        "wp": Wp.astype(bf),
        "bp": bp.astype(np.float32),
        "ident": np.eye(128, dtype=bf),
        "maskt": np.triu(np.ones((128, 128))).astype(bf),
    }
    return x, common


_NC_CACHE = None


def get_module():
    global _NC_CACHE
    if _NC_CACHE is None:
        _NC_CACHE = build_module()
    return _NC_CACHE


def run_with_results(inputs, **run_kwargs):
    x, common = host_prepare(inputs)
    nc = get_module()
    in_maps = [dict(common, x=np.ascontiguousarray(x[b])) for b in range(B)]
    res = run_bass_kernel_spmd(nc, in_maps, core_ids=list(range(N_CORES)), **run_kwargs)
    out = np.stack([res.results[b]["out"] for b in range(B)]).astype(np.float32)
    return out, res


def kernel(**inputs):
    return run_with_results(inputs)[0]


# revision 34
# speedup vs baseline: 1.2289x; 1.0212x over previous
"""Transformer block (LN -> causal MHA -> LN -> MLP, residuals) on 8 trn2 NeuronCores.

Data-parallel over batch: each core runs one [T, C] sequence independently
(no collectives). Matmul inputs are bf16 with fp32 PSUM accumulation;
layernorm, softmax and residuals stay fp32.

Host-side preprocessing folds the layernorm affine params into the adjacent
matmul weights, and folds the V bias through Wo, so the device kernel only
handles plain normalized activations.
"""

import math
import sys

for _p in ("/opt/trn_rl_repo", "/root/.axon_site/_ro/trn_rl_repo"):
    if _p not in sys.path:
        sys.path.append(_p)

import numpy as np
import ml_dtypes

import concourse.bass as bass
import concourse.mybir as mybir
import concourse.tile as tile
from concourse import bacc
from concourse.bass_utils import run_bass_kernel_spmd

B, T, C, H = 8, 1024, 1024, 16
D = C // H
NT = T // 128          # token tiles
NCK = C // 128         # contraction chunks over C
F32 = mybir.dt.float32
BF16 = mybir.dt.bfloat16
AF = mybir.ActivationFunctionType
N_CORES = 8


def _pieces(lo, hi, bound=512):
    """Split [lo, hi) at multiples of `bound` (PSUM bank boundaries)."""
    out = []
    a = lo
    while a < hi:
        b = min(hi, (a // bound + 1) * bound)
        out.append((a, b))
        a = b
    return out


def _layernorm_to_hT(nc, pools, src, hT, ident_sb, eps_sb):
    """src: [128, NT, C] f32 token-major. Writes hT [128, NCK, T] bf16 feature-major."""
    stat_pool, htok_pool, psT = pools
    for i in range(NT):
        xt = src[:, i, :]
        stats = stat_pool.tile([128, 2, 6], F32, tag="lnstats", name="lnstats")
        nc.vector.bn_stats(stats[:, 0, :], xt[:, 0:512])
        nc.vector.bn_stats(stats[:, 1, :], xt[:, 512:1024])
        mv = stat_pool.tile([128, 2], F32, tag="lnmv", name="lnmv")
        nc.vector.bn_aggr(mv, stats)
        std = stat_pool.tile([128, 1], F32, tag="lnstd", name="lnstd")
        nc.scalar.activation(std, mv[:, 1:2], AF.Sqrt, bias=eps_sb, scale=1.0)
        rstd = stat_pool.tile([128, 1], F32, tag="lnrstd", name="lnrstd")
        nc.vector.reciprocal(rstd, std)
        ht = htok_pool.tile([128, C], BF16, tag="htok", name="htok")
        nc.vector.tensor_scalar(
            out=ht, in0=xt, scalar1=mv[:, 0:1], scalar2=rstd,
            op0=mybir.AluOpType.subtract, op1=mybir.AluOpType.mult,
        )
        for cp in range(NCK // 2):
            ps = psT.tile([128, 2, 128], BF16, tag="pst", name="pst")
            nc.tensor.transpose(ps[:, 0, :], ht[:, (2 * cp) * 128:(2 * cp + 1) * 128], ident_sb)
            nc.tensor.transpose(ps[:, 1, :], ht[:, (2 * cp + 1) * 128:(2 * cp + 2) * 128], ident_sb)
            nc.vector.tensor_copy(hT[:, 2 * cp:2 * cp + 2, i * 128:(i + 1) * 128], ps)


def _build_body(tc, io, taps=None):
    nc = tc.nc
    x_d, wqk_d, wv_d, bqk_d, wo_d, bo_d, wfc_d, bfc_d, wp_d, bp_d, ident_d, maskt_d, out_d = io

    x_v = x_d.rearrange("(n p) c -> p n c", p=128)
    out_v = out_d.rearrange("(n p) c -> p n c", p=128)

    import contextlib
    est = contextlib.ExitStack()
    with est:
        const = est.enter_context(tc.tile_pool(name="const", bufs=1))
        ident_sb = const.tile([128, 128], BF16, tag="ident", name="ident_sb")
        nc.sync.dma_start(ident_sb, ident_d)
        maskt_sb = const.tile([128, 128], BF16, tag="maskt", name="maskt_sb")
        nc.sync.dma_start(maskt_sb, maskt_d)
        ones_sb = const.tile([1, 128], BF16, tag="ones", name="ones_sb")
        nc.vector.memset(ones_sb, 1.0)
        onesf_sb = const.tile([1, 64], F32, tag="onesf", name="onesf_sb")
        nc.vector.memset(onesf_sb, 1.0)
        eps_sb = const.tile([128, 1], F32, tag="eps", name="eps_sb")
        nc.vector.memset(eps_sb, 1e-5)
        bqk_sb = const.tile([128, 16], F32, tag="bqk", name="bqk_sb")
        nc.sync.dma_start(bqk_sb, bqk_d.rearrange("(n p) -> p n", p=128))
        bfc_sb = const.tile([128, 32], F32, tag="bfc", name="bfc_sb")
        nc.sync.dma_start(bfc_sb, bfc_d.rearrange("(n p) -> p n", p=128))
        bp_sb = const.tile([128, 8], F32, tag="bp", name="bp_sb")
        nc.sync.dma_start(bp_sb, bp_d.rearrange("(n p) -> p n", p=128))
        bo_sb = const.tile([1, C], BF16, tag="bo", name="bo_sb")
        nc.sync.dma_start(bo_sb, bo_d.rearrange("(a n) -> a n", a=1))

        ln_small = est.enter_context(tc.tile_pool(name="lnsmall", bufs=3))
        x_pool = est.enter_context(tc.tile_pool(name="xp", bufs=1))
        x_sb = x_pool.tile([128, NT, C], F32, tag="x", name="x_sb")
        yT_pool = est.enter_context(tc.tile_pool(name="ytp", bufs=1))
        yT = yT_pool.tile([128, NCK, T], BF16, tag="yT", name="yT")
        est_attn = est.enter_context(contextlib.ExitStack())
        attn_pool = est_attn.enter_context(tc.tile_pool(name="attnp", bufs=1))
        qkT = attn_pool.tile([128, 2 * NCK, T], BF16, tag="qkT", name="qkT")
        v_sb = attn_pool.tile([128, NT, H, D + 1], BF16, tag="v", name="v_sb")
        wv_sb = attn_pool.tile([128, NCK, C], BF16, tag="wv", name="wv_sb")
        nc.sync.dma_start(wv_sb, wv_d.rearrange("(n p) m -> p n m", p=128))
        nc.vector.memset(v_sb[:, :, :, D:D + 1], 1.0)

        # ---------------- phase 1: load x, LN1, transpose h ----------------
        with tc.tile_pool(name="hTp", bufs=1) as hT_pool, \
             tc.tile_pool(name="psT1", bufs=2, space="PSUM") as psT1, \
             tc.tile_pool(name="psA1", bufs=4, space="PSUM") as psA1, \
             tc.tile_pool(name="wq1", bufs=16) as wq_pool:
            hT = hT_pool.tile([128, NCK, T], BF16, tag="hT", name="hT")
            for i in range(NT):
                nc.sync.dma_start(x_sb[:, i, :], x_v[:, i, :])
            _layernorm_to_hT(nc, (ln_small, ln_small, psT1), x_sb, hT, ident_sb, eps_sb)

            # ---------------- phase 2: qkv projections ----------------
            # v token-major first (only needs per-token-tile hT, so PE warms up
            # while the qk feature groups' weights stream in)
            for ti in range(NT):
                for nsp in range(2):
                    ps = psA1.tile([128, 512], F32, tag="psqkv", name="psqkv")
                    for ck in range(NCK):
                        nc.tensor.matmul(
                            ps, lhsT=hT[:, ck, ti * 128:(ti + 1) * 128],
                            rhs=wv_sb[:, ck, nsp * 512:(nsp + 1) * 512],
                            start=(ck == 0), stop=(ck == NCK - 1),
                        )
                    nc.vector.tensor_copy(
                        v_sb[:, ti, nsp * 8:(nsp + 1) * 8, 0:D],
                        ps.rearrange("p (h d) -> p h d", h=8),
                    )
            # q,k feature-major: qkT[f, t] = sum_c Wqk[c, f] * hT[c, t]  (+bias via ACT)
            # Feature groups ordered so q-chunk / k-chunk pairs of the low heads
            # land first (heads can start scoring before all of qk is done).
            for fg in (0, 2, 1, 3):  # 512-wide feature groups over 2C
                wts = []
                for ck in range(NCK):
                    wt = wq_pool.tile([128, 512], BF16, tag="wqk", name="wqkt")
                    nc.sync.dma_start(wt, wqk_d[ck * 128:(ck + 1) * 128, fg * 512:(fg + 1) * 512])
                    wts.append(wt)
                for fl in range(4):
                    fn = fg * 4 + fl
                    for tsp in range(2):
                        ps = psA1.tile([128, 512], F32, tag="psqkv", name="psqkv")
                        for ck in range(NCK):
                            nc.tensor.matmul(
                                ps, lhsT=wts[ck][:, fl * 128:(fl + 1) * 128],
                                rhs=hT[:, ck, tsp * 512:(tsp + 1) * 512],
                                start=(ck == 0), stop=(ck == NCK - 1),
                            )
                        nc.scalar.activation(
                            qkT[:, fn, tsp * 512:(tsp + 1) * 512], ps,
                            AF.Identity, bias=bqk_sb[:, fn:fn + 1], scale=1.0,
                        )

        # ---------------- phase 3: attention (per head) ----------------
        # Heads are software-pipelined (scores for head h interleave with PV of
        # head h-1 so the in-order PE never stalls on the exp/mask chain).
        # y is stored UNNORMALIZED; rowsums (the ones-column of v_aug) are
        # staged into [16, T], reciprocal'd once, broadcast via GpSimd, and the
        # normalization happens as 8 head-pair muls at the end.
        with tc.tile_pool(name="ptp", bufs=2) as pt_pool, \
             tc.tile_pool(name="asml", bufs=2) as asml, \
             tc.tile_pool(name="psS", bufs=2, space="PSUM") as psS, \
             tc.tile_pool(name="psY", bufs=2, space="PSUM") as psY:
            inv_sqrt_c = 1.0 / math.sqrt(C)

            def scores_phase(h):
                po = 64 * (h % 2)
                hc = h // 2
                qT = qkT[po:po + 64, hc, :]
                kT = qkT[po:po + 64, NCK + hc, :]
                PT = pt_pool.tile([128, NT, T], BF16, tag="pt", name="PT")
                for j in range(NT):
                    lo = j * 128
                    ss = psS.tile([128, T], F32, tag="st", name="ss")
                    for (a, b) in _pieces(lo, T):
                        nc.tensor.matmul(
                            ss[:, a:b], lhsT=kT[:, lo:lo + 128], rhs=qT[:, a:b],
                            start=True, stop=True,
                        )
                    nc.scalar.activation(PT[:, j, lo:T], ss[:, lo:T], AF.Exp, scale=inv_sqrt_c)
                    nc.vector.tensor_mul(PT[:, j, lo:lo + 128], PT[:, j, lo:lo + 128], maskt_sb)
                return PT

            def pv_phase(h, PT):
                yps = psY.tile([65, T], F32, tag="y", name="yps")
                for j in range(NT):
                    lv = v_sb[:, j, h, :]
                    for (a, b) in _pieces(j * 128, T):
                        last = (j == min(NT - 1, (b - 1) // 128))
                        nc.tensor.matmul(
                            yps[:, a:b], lhsT=lv, rhs=PT[:, j, a:b],
                            start=(j == 0), stop=last, skip_group_check=True,
                        )
                # Drain PSUM right away: unnormalized y to SBUF bf16, rowsum
                # to a base-0 staging row (custom-DVE ops need base 0).
                yu = asml.tile([64, T], BF16, tag="yu", bufs=3, name="yu")
                nc.vector.tensor_copy(yu, yps[0:64, :])
                srow = asml.tile([1, 2, T], F32, tag="srow", bufs=3, name="srow")
                nc.vector.tensor_copy(srow[:, 0, :], yps[64:65, :])
                return yu, srow

            def epi_phase(h, yu, srow):
                # fast reciprocal -> bf16 -> PE ones-broadcast -> normalize into
                # packed yT (plain DVE ops may read base 0 and write base 64).
                po = 64 * (h % 2)
                hc = h // 2
                nc.vector.reciprocal_approx_fast(srow[:, 1, :], srow[:, 0, :])
                rbf = asml.tile([1, T], BF16, tag="rbf", name="rbf")
                nc.vector.tensor_copy(rbf, srow[:, 1, :])
                rbps = psS.tile([64, T], F32, tag="st", name="rbps")
                for (a, b) in ((0, 512), (512, 1024)):
                    nc.tensor.matmul(rbps[:, a:b], lhsT=ones_sb[0:1, 0:64], rhs=rbf[0:1, a:b],
                                     start=True, stop=True)
                rbs = asml.tile([64, T], F32, tag="rbs", name="rbs")
                nc.scalar.copy(rbs, rbps)
                if taps is not None:
                    nc.sync.dma_start(taps["sums"][h:h + 1, :], srow[:, 0, :])
                    nc.sync.dma_start(taps["recips"][h:h + 1, :], srow[:, 1, :])
                    nc.sync.dma_start(taps["rbsrow"][h:h + 1, :], rbs[0:1, :])
                nc.vector.tensor_mul(yT[po:po + 64, hc, :], yu, rbs)

            # 3-stage pipeline: scores(h) | PV(h-1) | epilogue(h-2). The PE
            # never waits on the reciprocal chain: by the time the tiny
            # broadcast matmuls of head h-2 reach the in-order PE queue their
            # inputs have long been ready.
            pts = {}
            pvres = {}
            for h in range(H):
                pts[h] = scores_phase(h)
                if h - 1 >= 0:
                    pvres[h - 1] = pv_phase(h - 1, pts.pop(h - 1))
                if h - 2 >= 0:
                    epi_phase(h - 2, *pvres.pop(h - 2))
            pvres[H - 1] = pv_phase(H - 1, pts.pop(H - 1))
            epi_phase(H - 2, *pvres.pop(H - 2))
            epi_phase(H - 1, *pvres.pop(H - 1))

        est_attn.close()  # free qkT/v/wv space
        x2 = x_sb  # attention residual is written in place

        if taps is not None:
            nc.sync.dma_start(taps["yT"], yT)

        # ---------------- phase 4: attention out-proj + residual ----------------
        with tc.tile_pool(name="wop", bufs=1) as wo_pool, \
             tc.tile_pool(name="psA2", bufs=4, space="PSUM") as psA2:
            wo_sb = wo_pool.tile([128, NCK, C], BF16, tag="wo", name="wo_sb")
            nc.sync.dma_start(wo_sb, wo_d.rearrange("(n p) m -> p n m", p=128))
            for ti in range(NT):
                for nsp in range(2):
                    ps = psA2.tile([128, 512], F32, tag="pswo", name="pswo")
                    for cn in range(NCK):
                        nc.tensor.matmul(
                            ps, lhsT=yT[:, cn, ti * 128:(ti + 1) * 128],
                            rhs=wo_sb[:, cn, nsp * 512:(nsp + 1) * 512],
                            start=(cn == 0), stop=False,
                        )
                    nc.tensor.matmul(ps, lhsT=ones_sb[0:1, 0:128],
                                     rhs=bo_sb[0:1, nsp * 512:(nsp + 1) * 512],
                                     start=False, stop=True)
                    nc.vector.tensor_add(
                        x2[:, ti, nsp * 512:(nsp + 1) * 512], ps,
                        x_sb[:, ti, nsp * 512:(nsp + 1) * 512],
                    )

        # ---------------- phase 5/6: LN2 + FC(gelu) ----------------
        mlp_pool = est.enter_context(tc.tile_pool(name="mlpp", bufs=1))
        mT = mlp_pool.tile([128, 4 * NCK, T], BF16, tag="mT", name="mT")
        outT = mlp_pool.tile([128, NCK, T], BF16, tag="outT", name="outT")
        with tc.tile_pool(name="h2Tp", bufs=1) as h2T_pool, \
             tc.tile_pool(name="psT2", bufs=2, space="PSUM") as psT2, \
             tc.tile_pool(name="psA3", bufs=4, space="PSUM") as psA3, \
             tc.tile_pool(name="wf1", bufs=16) as wf_pool:
            h2T = h2T_pool.tile([128, NCK, T], BF16, tag="h2T", name="h2T")
            _layernorm_to_hT(nc, (ln_small, ln_small, psT2), x2, h2T, ident_sb, eps_sb)
            for fg in range(8):  # 512-wide feature groups over 4C
                wts = []
                for ck in range(NCK):
                    wt = wf_pool.tile([128, 512], BF16, tag="wfc", name="wfct")
                    nc.sync.dma_start(wt, wfc_d[ck * 128:(ck + 1) * 128, fg * 512:(fg + 1) * 512])
                    wts.append(wt)
                for fl in range(4):
                    fn = fg * 4 + fl
                    for tsp in range(2):
                        ps = psA3.tile([128, 512], F32, tag="psfc", name="psfc")
                        for ck in range(NCK):
                            nc.tensor.matmul(
                                ps, lhsT=wts[ck][:, fl * 128:(fl + 1) * 128],
                                rhs=h2T[:, ck, tsp * 512:(tsp + 1) * 512],
                                start=(ck == 0), stop=(ck == NCK - 1),
                            )
                        nc.scalar.activation(
                            mT[:, fn, tsp * 512:(tsp + 1) * 512], ps,
                            AF.Gelu_apprx_tanh, bias=bfc_sb[:, fn:fn + 1], scale=1.0,
                        )

        # ---------------- phase 7: Wp (feature-major out) ----------------
        with tc.tile_pool(name="wpp", bufs=6) as wp_pool, \
             tc.tile_pool(name="psW", bufs=4, space="PSUM") as psW, \
             tc.tile_pool(name="psT3", bufs=2, space="PSUM") as psT3, \
             tc.tile_pool(name="outp", bufs=2) as out_pool:
            for cg in range(4):  # output feature groups of 256
                pss = [[psW.tile([128, 512], F32, tag="pswp", name="pswp")
                        for _ in range(2)] for _ in range(2)]
                for kn in range(4 * NCK):
                    wt = wp_pool.tile([128, 256], BF16, tag="wp", name="wpt")
                    nc.scalar.dma_start(wt, wp_d[kn * 128:(kn + 1) * 128, cg * 256:(cg + 1) * 256])
                    for cl in range(2):
                        for tsp in range(2):
                            nc.tensor.matmul(
                                pss[cl][tsp], lhsT=wt[:, cl * 128:(cl + 1) * 128],
                                rhs=mT[:, kn, tsp * 512:(tsp + 1) * 512],
                                start=(kn == 0), stop=(kn == 4 * NCK - 1),
                            )
                for cl in range(2):
                    cn = cg * 2 + cl
                    for tsp in range(2):
                        nc.scalar.activation(
                            outT[:, cn, tsp * 512:(tsp + 1) * 512], pss[cl][tsp],
                            AF.Identity, bias=bp_sb[:, cn:cn + 1], scale=1.0,
                        )
            # ---------------- phase 8: transpose back + residual + store ----------------
            for ti in range(NT):
                outt = out_pool.tile([128, C], F32, tag="osb", name="outt")
                for cg in range(2):
                    ps4 = psT3.tile([128, 4, 128], BF16, tag="pst3", name="ps4")
                    for cl in range(4):
                        cj = cg * 4 + cl
                        nc.tensor.transpose(ps4[:, cl, :], outT[:, cj, ti * 128:(ti + 1) * 128], ident_sb)
                    nc.vector.tensor_add(
                        outt[:, cg * 512:(cg + 1) * 512].rearrange("p (a b) -> p a b", a=4),
                        ps4,
                        x2[:, ti, cg * 512:(cg + 1) * 512].rearrange("p (a b) -> p a b", a=4),
                    )
                nc.sync.dma_start(out_v[:, ti, :], outt)


def build_module(debug_taps=False):
    nc = bacc.Bacc("TRN2", target_bir_lowering=False, debug=False)

    def din(name, shape, dtype):
        return nc.dram_tensor(name, list(shape), dtype, kind="ExternalInput").ap()

    taps = None
    if debug_taps:
        taps = {
            "sums": nc.dram_tensor("dbg_sums", [H, T], F32, kind="ExternalOutput").ap(),
            "recips": nc.dram_tensor("dbg_recips", [H, T], F32, kind="ExternalOutput").ap(),
            "rbsrow": nc.dram_tensor("dbg_rbsrow", [H, T], F32, kind="ExternalOutput").ap(),
            "yT": nc.dram_tensor("dbg_yT", [128, NCK, T], BF16, kind="ExternalOutput").ap(),
        }

    io = (
        din("x", (T, C), F32),
        din("wqk", (C, 2 * C), BF16),
        din("wv", (C, C), BF16),
        din("bqk", (2 * C,), F32),
        din("wo", (C, C), BF16),
        din("bo", (C,), BF16),
        din("wfc", (C, 4 * C), BF16),
        din("bfc", (4 * C,), F32),
        din("wp", (4 * C, C), BF16),
        din("bp", (C,), F32),
        din("ident", (128, 128), BF16),
        din("maskt", (128, 128), BF16),
        nc.dram_tensor("out", [T, C], F32, kind="ExternalOutput").ap(),
    )
    with tile.TileContext(nc) as tc:
        _build_body(tc, io, taps=taps)
    nc.compile()
    return nc


def host_prepare(inputs):
    """Fold LN affine params / v-bias into weights; cast matmul weights to bf16."""
    bf = ml_dtypes.bfloat16
    x = np.asarray(inputs["x"], np.float32)
    Wqkv = np.asarray(inputs["Wqkv"], np.float64)
    bqkv = np.asarray(inputs["bqkv"], np.float64)
    Wo = np.asarray(inputs["Wo"], np.float64)
    bo = np.asarray(inputs["bo"], np.float64)
    ln1_w = np.asarray(inputs["ln1_w"], np.float64)
    ln1_b = np.asarray(inputs["ln1_b"], np.float64)
    ln2_w = np.asarray(inputs["ln2_w"], np.float64)
    ln2_b = np.asarray(inputs["ln2_b"], np.float64)
    Wfc = np.asarray(inputs["Wfc"], np.float64)
    bfc = np.asarray(inputs["bfc"], np.float64)
    Wp = np.asarray(inputs["Wp"], np.float64)
    bp = np.asarray(inputs["bp"], np.float64)

    Wqkv_f = ln1_w[:, None] * Wqkv
    bqkv_f = bqkv + ln1_b @ Wqkv
    bo_f = bo + bqkv_f[2 * C:] @ Wo
    Wfc_f = ln2_w[:, None] * Wfc
    bfc_f = bfc + ln2_b @ Wfc

    common = {
        "wqk": Wqkv_f[:, :2 * C].astype(bf),
        "wv": Wqkv_f[:, 2 * C:].astype(bf),
        "bqk": bqkv_f[:2 * C].astype(np.float32),
        "wo": Wo.astype(bf),
        "bo": bo_f.astype(bf),
        "wfc": Wfc_f.astype(bf),
        "bfc": bfc_f.astype(np.float32),
        "wp": Wp.astype(bf),
        "bp": bp.astype(np.float32),
        "ident": np.eye(128, dtype=bf),
        "maskt": np.triu(np.ones((128, 128))).astype(bf),
    }
    return x, common


_NC_CACHE = None


def get_module():
    global _NC_CACHE
    if _NC_CACHE is None:
        _NC_CACHE = build_module()
    return _NC_CACHE


def run_with_results(inputs, **run_kwargs):
    x, common = host_prepare(inputs)
    nc = get_module()
    in_maps = [dict(common, x=np.ascontiguousarray(x[b])) for b in range(B)]
    res = run_bass_kernel_spmd(nc, in_maps, core_ids=list(range(N_CORES)), **run_kwargs)
    out = np.stack([res.results[b]["out"] for b in range(B)]).astype(np.float32)
    return out, res


def kernel(**inputs):
    return run_with_results(inputs)[0]


# revision 50
# speedup vs baseline: 1.3738x; 1.1179x over previous
"""Transformer block (LN -> causal MHA -> LN -> MLP, residuals) on 8 trn2 NeuronCores.

Data-parallel over batch: each core runs one [T, C] sequence independently
(no collectives). Matmul inputs are bf16 with fp32 PSUM accumulation;
layernorm, softmax and residuals stay fp32.

Host-side preprocessing folds the layernorm affine params into the adjacent
matmul weights, and folds the V bias through Wo, so the device kernel only
handles plain normalized activations.
"""

import math
import sys

for _p in ("/opt/trn_rl_repo", "/root/.axon_site/_ro/trn_rl_repo"):
    if _p not in sys.path:
        sys.path.append(_p)

import numpy as np
import ml_dtypes

import concourse.bass as bass
import concourse.mybir as mybir
import concourse.tile as tile
from concourse import bacc
from concourse.bass_utils import run_bass_kernel_spmd

B, T, C, H = 8, 1024, 1024, 16
D = C // H
NT = T // 128          # token tiles
NCK = C // 128         # contraction chunks over C
F32 = mybir.dt.float32
BF16 = mybir.dt.bfloat16
AF = mybir.ActivationFunctionType
N_CORES = 8


def _pieces(lo, hi, bound=512):
    """Split [lo, hi) at multiples of `bound` (PSUM bank boundaries)."""
    out = []
    a = lo
    while a < hi:
        b = min(hi, (a // bound + 1) * bound)
        out.append((a, b))
        a = b
    return out


def _layernorm_to_hT(nc, pools, src, hT, ident_sb, eps_sb):
    """src: [128, NT, C] f32 token-major. Writes hT [128, NCK, T] bf16 feature-major."""
    stat_pool, htok_pool, psT = pools
    for i in range(NT):
        xt = src[:, i, :]
        stats = stat_pool.tile([128, 2, 6], F32, tag="lnstats", name="lnstats")
        nc.vector.bn_stats(stats[:, 0, :], xt[:, 0:512])
        nc.vector.bn_stats(stats[:, 1, :], xt[:, 512:1024])
        mv = stat_pool.tile([128, 2], F32, tag="lnmv", name="lnmv")
        nc.vector.bn_aggr(mv, stats)
        std = stat_pool.tile([128, 1], F32, tag="lnstd", name="lnstd")
        nc.scalar.activation(std, mv[:, 1:2], AF.Sqrt, bias=eps_sb, scale=1.0)
        rstd = stat_pool.tile([128, 1], F32, tag="lnrstd", name="lnrstd")
        nc.vector.reciprocal(rstd, std)
        ht = htok_pool.tile([128, C], BF16, tag="htok", name="htok")
        nc.vector.tensor_scalar(
            out=ht, in0=xt, scalar1=mv[:, 0:1], scalar2=rstd,
            op0=mybir.AluOpType.subtract, op1=mybir.AluOpType.mult,
        )
        for cp in range(NCK // 2):
            ps = psT.tile([128, 2, 128], BF16, tag="pst", name="pst")
            nc.tensor.transpose(ps[:, 0, :], ht[:, (2 * cp) * 128:(2 * cp + 1) * 128], ident_sb)
            nc.tensor.transpose(ps[:, 1, :], ht[:, (2 * cp + 1) * 128:(2 * cp + 2) * 128], ident_sb)
            nc.vector.tensor_copy(hT[:, 2 * cp:2 * cp + 2, i * 128:(i + 1) * 128], ps)


def _build_body(tc, io, taps=None):
    nc = tc.nc
    x_d, wqk_d, wv_d, bqk_d, wo_d, bo_d, wfc_d, bfc_d, wp_d, bp_d, ident_d, maskt_d, out_d = io

    x_v = x_d.rearrange("(n p) c -> p n c", p=128)
    out_v = out_d.rearrange("(n p) c -> p n c", p=128)

    import contextlib
    est = contextlib.ExitStack()
    with est:
        # x tiles first: everything else on the DMA queues can wait, LN1 cannot.
        x_pool = est.enter_context(tc.tile_pool(name="xp", bufs=1))
        x_sb = x_pool.tile([128, NT, C], F32, tag="x", name="x_sb")
        for i in range(NT):
            nc.sync.dma_start(x_sb[:, i, :], x_v[:, i, :])

        const = est.enter_context(tc.tile_pool(name="const", bufs=1))
        ident_sb = const.tile([128, 128], BF16, tag="ident", name="ident_sb")
        nc.sync.dma_start(ident_sb, ident_d)
        maskt_sb = const.tile([128, 128], BF16, tag="maskt", name="maskt_sb")
        nc.sync.dma_start(maskt_sb, maskt_d)
        ones_sb = const.tile([1, 128], BF16, tag="ones", name="ones_sb")
        nc.vector.memset(ones_sb, 1.0)
        onesf_sb = const.tile([1, 64], F32, tag="onesf", name="onesf_sb")
        nc.vector.memset(onesf_sb, 1.0)
        eps_sb = const.tile([128, 1], F32, tag="eps", name="eps_sb")
        nc.vector.memset(eps_sb, 1e-5)
        bqk_sb = const.tile([128, 16], F32, tag="bqk", name="bqk_sb")
        nc.sync.dma_start(bqk_sb, bqk_d.rearrange("(n p) -> p n", p=128))
        bfc_sb = const.tile([128, 32], F32, tag="bfc", name="bfc_sb")
        nc.sync.dma_start(bfc_sb, bfc_d.rearrange("(n p) -> p n", p=128))
        bp_sb = const.tile([128, 8], F32, tag="bp", name="bp_sb")
        nc.sync.dma_start(bp_sb, bp_d.rearrange("(n p) -> p n", p=128))
        bo_sb = const.tile([1, C], BF16, tag="bo", name="bo_sb")
        nc.sync.dma_start(bo_sb, bo_d.rearrange("(a n) -> a n", a=1))

        ln_small = est.enter_context(tc.tile_pool(name="lnsmall", bufs=3))
        yT_pool = est.enter_context(tc.tile_pool(name="ytp", bufs=1))
        yT = yT_pool.tile([128, NCK, T], BF16, tag="yT", name="yT")
        est_attn = est.enter_context(contextlib.ExitStack())
        attn_pool = est_attn.enter_context(tc.tile_pool(name="attnp", bufs=1))
        # k feature-major, two heads packed per 128-row chunk (as produced).
        kT_sb = attn_pool.tile([128, NCK, T], BF16, tag="kT", name="kT_sb")
        # q stored per-head: head h occupies partitions [64*(h%2), +64) of its
        # chunk, the other 64 rows stay ZERO. The scores matmul can then use
        # the full 128-row k chunk as lhsT (junk rows hit zero q rows), keeping
        # the PE at K=128 so the HAM clock gate sees a busy array (K=64
        # matmuls left the whole attention phase throttled to 1.2 GHz).
        qT2 = attn_pool.tile([128, H, T], BF16, tag="qT2", name="qT2")
        v_sb = attn_pool.tile([128, NT, H, D + 1], BF16, tag="v", name="v_sb")
        nc.vector.memset(v_sb[:, :, :, D:D + 1], 1.0)

        # ---------------- phase 1: load x, LN1, transpose h ----------------
        with tc.tile_pool(name="hTp", bufs=1) as hT_pool, \
             tc.tile_pool(name="psT1", bufs=2, space="PSUM") as psT1, \
             tc.tile_pool(name="psA1", bufs=6, space="PSUM") as psA1, \
             tc.tile_pool(name="wq1", bufs=16) as wq_pool:
            hT = hT_pool.tile([128, NCK, T], BF16, tag="hT", name="hT")
            wv_sb = hT_pool.tile([128, NCK, C], BF16, tag="wv", name="wv_sb")
            nc.sync.dma_start(wv_sb, wv_d.rearrange("(n p) m -> p n m", p=128))
            _layernorm_to_hT(nc, (ln_small, ln_small, psT1), x_sb, hT, ident_sb, eps_sb)

            # ---------------- phase 2: qkv projections ----------------
            # v token-major first (only needs per-token-tile hT, so PE warms up
            # while the qk feature groups' weights stream in)
            for ti in range(NT):
                for nsp in range(2):
                    ps = psA1.tile([128, 512], F32, tag="psqkv", name="psqkv")
                    for ck in range(NCK):
                        nc.tensor.matmul(
                            ps, lhsT=hT[:, ck, ti * 128:(ti + 1) * 128],
                            rhs=wv_sb[:, ck, nsp * 512:(nsp + 1) * 512],
                            start=(ck == 0), stop=(ck == NCK - 1),
                        )
                    nc.vector.tensor_copy(
                        v_sb[:, ti, nsp * 8:(nsp + 1) * 8, 0:D],
                        ps.rearrange("p (h d) -> p h d", h=8),
                    )
            # q,k feature-major: qkT[f, t] = sum_c Wqk[c, f] * hT[c, t]  (+bias via ACT)
            # Feature groups ordered so q-chunk / k-chunk pairs of the low heads
            # land first (heads can start scoring before all of qk is done).
            for fg in (0, 2, 1, 3):  # 512-wide feature groups over 2C
                wts = []
                for ck in range(NCK):
                    wt = wq_pool.tile([128, 512], BF16, tag="wqk", name="wqkt")
                    nc.sync.dma_start(wt, wqk_d[ck * 128:(ck + 1) * 128, fg * 512:(fg + 1) * 512])
                    wts.append(wt)
                for fl in range(4):
                    fn = fg * 4 + fl
                    for tsp in range(2):
                        ps = psA1.tile([128, 512], F32, tag="psqkv", name="psqkv")
                        for ck in range(NCK):
                            nc.tensor.matmul(
                                ps, lhsT=wts[ck][:, fl * 128:(fl + 1) * 128],
                                rhs=hT[:, ck, tsp * 512:(tsp + 1) * 512],
                                start=(ck == 0), stop=(ck == NCK - 1),
                            )
                        sl = slice(tsp * 512, (tsp + 1) * 512)
                        if fn < NCK:  # q chunk -> per-head halves of qT2
                            nc.scalar.activation(
                                qT2[0:64, 2 * fn, sl], ps[0:64, :],
                                AF.Identity, bias=bqk_sb[0:64, fn:fn + 1], scale=1.0,
                            )
                            nc.scalar.activation(
                                qT2[64:128, 2 * fn + 1, sl], ps[64:128, :],
                                AF.Identity, bias=bqk_sb[64:128, fn:fn + 1], scale=1.0,
                            )
                            # zero the unused halves (replaces a 17us DVE memset
                            # that used to block LN1 at kernel start)
                            nc.scalar.mul(qT2[64:128, 2 * fn, sl], ps[64:128, :], 0.0)
                            nc.scalar.mul(qT2[0:64, 2 * fn + 1, sl], ps[0:64, :], 0.0)
                        else:  # k chunk
                            nc.scalar.activation(
                                kT_sb[:, fn - NCK, sl], ps,
                                AF.Identity, bias=bqk_sb[:, fn:fn + 1], scale=1.0,
                            )

        # Prefetch Wo into the space wv_sb just released; the 2MB DMA runs
        # behind the attention phase instead of stalling its epilogue.
        wo_pool = est_attn.enter_context(tc.tile_pool(name="wop", bufs=1))
        wo_sb = wo_pool.tile([128, NCK, C], BF16, tag="wo", name="wo_sb")
        nc.sync.dma_start(wo_sb, wo_d.rearrange("(n p) m -> p n m", p=128))

        # ---------------- phase 3: attention (per head) ----------------
        with tc.tile_pool(name="ptp", bufs=2) as pt_pool, \
             tc.tile_pool(name="asml", bufs=2) as asml, \
             tc.tile_pool(name="psS", bufs=2, space="PSUM") as psS, \
             tc.tile_pool(name="psY", bufs=2, space="PSUM") as psY:
            inv_sqrt_c = 1.0 / math.sqrt(C)

            def scores_phase(h):
                hc = h // 2
                qT = qT2[:, h, :]               # zero-padded to 128 rows
                kT = kT_sb[:, hc, :]            # full chunk; junk rows hit q zeros
                PT = pt_pool.tile([128, NT, T], BF16, tag="pt", name="PT")
                for j in range(NT):
                    lo = j * 128
                    ss = psS.tile([128, T], F32, tag="st", name="ss")
                    for (a, b) in _pieces(lo, T):
                        nc.tensor.matmul(
                            ss[:, a:b], lhsT=kT[:, lo:lo + 128], rhs=qT[:, a:b],
                            start=True, stop=True,
                        )
                    nc.scalar.activation(PT[:, j, lo:T], ss[:, lo:T], AF.Exp, scale=inv_sqrt_c)
                    nc.vector.tensor_mul(PT[:, j, lo:lo + 128], PT[:, j, lo:lo + 128], maskt_sb)
                return PT

            def pv_phase(h, PT):
                yps = psY.tile([65, T], F32, tag="y", name="yps")
                for j in range(NT):
                    lv = v_sb[:, j, h, :]
                    for (a, b) in _pieces(j * 128, T):
                        last = (j == min(NT - 1, (b - 1) // 128))
                        nc.tensor.matmul(
                            yps[:, a:b], lhsT=lv, rhs=PT[:, j, a:b],
                            start=(j == 0), stop=last, skip_group_check=True,
                        )
                # Drain PSUM right away: unnormalized y to SBUF bf16, rowsum
                # to a base-0 staging row (custom-DVE ops need base 0).
                yu = asml.tile([64, T], BF16, tag="yu", bufs=3, name="yu")
                nc.vector.tensor_copy(yu, yps[0:64, :])
                srow = asml.tile([1, 2, T], F32, tag="srow", bufs=2, name="srow")
                nc.vector.tensor_copy(srow[:, 0, :], yps[64:65, :])
                return yu, srow

            def epi_phase(h, yu, srow):
                # fast reciprocal -> bf16 -> PE ones-broadcast -> normalize into
                # packed yT (plain DVE ops may read base 0 and write base 64).
                po = 64 * (h % 2)
                hc = h // 2
                nc.vector.reciprocal_approx_fast(srow[:, 1, :], srow[:, 0, :])
                rbf = asml.tile([1, T], BF16, tag="rbf", name="rbf")
                nc.vector.tensor_copy(rbf, srow[:, 1, :])
                rbps = psS.tile([64, T], F32, tag="st", name="rbps")
                for (a, b) in ((0, 512), (512, 1024)):
                    nc.tensor.matmul(rbps[:, a:b], lhsT=ones_sb[0:1, 0:64], rhs=rbf[0:1, a:b],
                                     start=True, stop=True)
                rbs = asml.tile([64, T], F32, tag="rbs", name="rbs")
                nc.scalar.copy(rbs, rbps)
                if taps is not None:
                    nc.sync.dma_start(taps["sums"][h:h + 1, :], srow[:, 0, :])
                    nc.sync.dma_start(taps["recips"][h:h + 1, :], srow[:, 1, :])
                    nc.sync.dma_start(taps["rbsrow"][h:h + 1, :], rbs[0:1, :])
                nc.vector.tensor_mul(yT[po:po + 64, hc, :], yu, rbs)

            # 3-stage pipeline: scores(h) | PV(h-1) | epilogue(h-2). The PE
            # never waits on the reciprocal chain: by the time the tiny
            # broadcast matmuls of head h-2 reach the in-order PE queue their
            # inputs have long been ready.
            pts = {}
            pvres = {}
            for h in range(H):
                pts[h] = scores_phase(h)
                if h - 1 >= 0:
                    pvres[h - 1] = pv_phase(h - 1, pts.pop(h - 1))
                if h - 2 >= 0:
                    epi_phase(h - 2, *pvres.pop(h - 2))
            pvres[H - 1] = pv_phase(H - 1, pts.pop(H - 1))
            epi_phase(H - 2, *pvres.pop(H - 2))
            epi_phase(H - 1, *pvres.pop(H - 1))

        x2 = x_sb  # attention residual is written in place

        if taps is not None:
            nc.sync.dma_start(taps["yT"], yT)

        # ---------------- phase 4: attention out-proj + residual ----------------
        with tc.tile_pool(name="psA2", bufs=4, space="PSUM") as psA2:
            for ti in range(NT):
                for nsp in range(2):
                    ps = psA2.tile([128, 512], F32, tag="pswo", name="pswo")
                    for cn in range(NCK):
                        nc.tensor.matmul(
                            ps, lhsT=yT[:, cn, ti * 128:(ti + 1) * 128],
                            rhs=wo_sb[:, cn, nsp * 512:(nsp + 1) * 512],
                            start=(cn == 0), stop=False,
                        )
                    nc.tensor.matmul(ps, lhsT=ones_sb[0:1, 0:128],
                                     rhs=bo_sb[0:1, nsp * 512:(nsp + 1) * 512],
                                     start=False, stop=True)
                    nc.vector.tensor_add(
                        x2[:, ti, nsp * 512:(nsp + 1) * 512], ps,
                        x_sb[:, ti, nsp * 512:(nsp + 1) * 512],
                    )

        est_attn.close()  # free kT/qT2/v/wo space before MLP tensors
        # ---------------- phase 5/6: LN2 + FC(gelu) ----------------
        mlp_pool = est.enter_context(tc.tile_pool(name="mlpp", bufs=1))
        mT = mlp_pool.tile([128, 4 * NCK, T], BF16, tag="mT", name="mT")
        outT = mlp_pool.tile([128, NCK, T], BF16, tag="outT", name="outT")
        with tc.tile_pool(name="h2Tp", bufs=1) as h2T_pool, \
             tc.tile_pool(name="psT2", bufs=2, space="PSUM") as psT2, \
             tc.tile_pool(name="psA3", bufs=4, space="PSUM") as psA3, \
             tc.tile_pool(name="wf1", bufs=16) as wf_pool:
            h2T = h2T_pool.tile([128, NCK, T], BF16, tag="h2T", name="h2T")
            _layernorm_to_hT(nc, (ln_small, ln_small, psT2), x2, h2T, ident_sb, eps_sb)
            for fg in range(8):  # 512-wide feature groups over 4C
                wts = []
                for ck in range(NCK):
                    wt = wf_pool.tile([128, 512], BF16, tag="wfc", name="wfct")
                    nc.sync.dma_start(wt, wfc_d[ck * 128:(ck + 1) * 128, fg * 512:(fg + 1) * 512])
                    wts.append(wt)
                for fl in range(4):
                    fn = fg * 4 + fl
                    for tsp in range(2):
                        ps = psA3.tile([128, 512], F32, tag="psfc", name="psfc")
                        for ck in range(NCK):
                            nc.tensor.matmul(
                                ps, lhsT=wts[ck][:, fl * 128:(fl + 1) * 128],
                                rhs=h2T[:, ck, tsp * 512:(tsp + 1) * 512],
                                start=(ck == 0), stop=(ck == NCK - 1),
                            )
                        nc.scalar.activation(
                            mT[:, fn, tsp * 512:(tsp + 1) * 512], ps,
                            AF.Gelu_apprx_tanh, bias=bfc_sb[:, fn:fn + 1], scale=1.0,
                        )

        # ---------------- phase 7: Wp (feature-major out) ----------------
        with tc.tile_pool(name="wpp", bufs=6) as wp_pool, \
             tc.tile_pool(name="psW", bufs=4, space="PSUM") as psW, \
             tc.tile_pool(name="psT3", bufs=2, space="PSUM") as psT3, \
             tc.tile_pool(name="outp", bufs=8) as out_pool:

            def out_half(half, outts):
                # transpose-back + residual for output column half `half`
                # (needs Wp groups 2*half..2*half+1 only); emitted mid-Wp so
                # these PE transposes run while the array is dense and warm
                # instead of in a cold tail after the last Wp matmul.
                for ti in range(NT):
                    if half == 0:
                        outts.append(out_pool.tile([128, C], F32, tag="osb", name="outt"))
                    outt = outts[ti]
                    ps4 = psT3.tile([128, 4, 128], BF16, tag="pst3", name="ps4")
                    for cl in range(4):
                        cj = half * 4 + cl
                        nc.tensor.transpose(ps4[:, cl, :], outT[:, cj, ti * 128:(ti + 1) * 128], ident_sb)
                    nc.vector.tensor_add(
                        outt[:, half * 512:(half + 1) * 512].rearrange("p (a b) -> p a b", a=4),
                        ps4,
                        x2[:, ti, half * 512:(half + 1) * 512].rearrange("p (a b) -> p a b", a=4),
                    )
                    if half == 1:
                        nc.sync.dma_start(out_v[:, ti, :], outt)

            outts = []
            for cg in range(4):  # output feature groups of 256
                pss = [[psW.tile([128, 512], F32, tag="pswp", name="pswp")
                        for _ in range(2)] for _ in range(2)]
                for kn in range(4 * NCK):
                    wt = wp_pool.tile([128, 256], BF16, tag="wp", name="wpt")
                    nc.scalar.dma_start(wt, wp_d[kn * 128:(kn + 1) * 128, cg * 256:(cg + 1) * 256])
                    for cl in range(2):
                        for tsp in range(2):
                            nc.tensor.matmul(
                                pss[cl][tsp], lhsT=wt[:, cl * 128:(cl + 1) * 128],
                                rhs=mT[:, kn, tsp * 512:(tsp + 1) * 512],
                                start=(kn == 0), stop=(kn == 4 * NCK - 1),
                            )
                for cl in range(2):
                    cn = cg * 2 + cl
                    for tsp in range(2):
                        nc.scalar.activation(
                            outT[:, cn, tsp * 512:(tsp + 1) * 512], pss[cl][tsp],
                            AF.Identity, bias=bp_sb[:, cn:cn + 1], scale=1.0,
                        )
                if cg == 1:
                    out_half(0, outts)
                if cg == 3:
                    out_half(1, outts)


def build_module(debug_taps=False):
    nc = bacc.Bacc("TRN2", target_bir_lowering=False, debug=False)

    def din(name, shape, dtype):
        return nc.dram_tensor(name, list(shape), dtype, kind="ExternalInput").ap()

    taps = None
    if debug_taps:
        taps = {
            "sums": nc.dram_tensor("dbg_sums", [H, T], F32, kind="ExternalOutput").ap(),
            "recips": nc.dram_tensor("dbg_recips", [H, T], F32, kind="ExternalOutput").ap(),
            "rbsrow": nc.dram_tensor("dbg_rbsrow", [H, T], F32, kind="ExternalOutput").ap(),
            "yT": nc.dram_tensor("dbg_yT", [128, NCK, T], BF16, kind="ExternalOutput").ap(),
        }

    io = (
        din("x", (T, C), F32),
        din("wqk", (C, 2 * C), BF16),
        din("wv", (C, C), BF16),
        din("bqk", (2 * C,), F32),
        din("wo", (C, C), BF16),
        din("bo", (C,), BF16),
        din("wfc", (C, 4 * C), BF16),
        din("bfc", (4 * C,), F32),
        din("wp", (4 * C, C), BF16),
        din("bp", (C,), F32),
        din("ident", (128, 128), BF16),
        din("maskt", (128, 128), BF16),
        nc.dram_tensor("out", [T, C], F32, kind="ExternalOutput").ap(),
    )
    with tile.TileContext(nc) as tc:
        _build_body(tc, io, taps=taps)
    nc.compile()
    return nc


def host_prepare(inputs):
    """Fold LN affine params / v-bias into weights; cast matmul weights to bf16."""
    bf = ml_dtypes.bfloat16
    x = np.asarray(inputs["x"], np.float32)
    Wqkv = np.asarray(inputs["Wqkv"], np.float64)
    bqkv = np.asarray(inputs["bqkv"], np.float64)
    Wo = np.asarray(inputs["Wo"], np.float64)
    bo = np.asarray(inputs["bo"], np.float64)
    ln1_w = np.asarray(inputs["ln1_w"], np.float64)
    ln1_b = np.asarray(inputs["ln1_b"], np.float64)
    ln2_w = np.asarray(inputs["ln2_w"], np.float64)
    ln2_b = np.asarray(inputs["ln2_b"], np.float64)
    Wfc = np.asarray(inputs["Wfc"], np.float64)
    bfc = np.asarray(inputs["bfc"], np.float64)
    Wp = np.asarray(inputs["Wp"], np.float64)
    bp = np.asarray(inputs["bp"], np.float64)

    Wqkv_f = ln1_w[:, None] * Wqkv
    bqkv_f = bqkv + ln1_b @ Wqkv
    bo_f = bo + bqkv_f[2 * C:] @ Wo
    Wfc_f = ln2_w[:, None] * Wfc
    bfc_f = bfc + ln2_b @ Wfc

    common = {
        "wqk": Wqkv_f[:, :2 * C].astype(bf),
        "wv": Wqkv_f[:, 2 * C:].astype(bf),
        "bqk": bqkv_f[:2 * C].astype(np.float32),
        "wo": Wo.astype(bf),
        "bo": bo_f.astype(bf),
        "wfc": Wfc_f.astype(bf),
        "bfc": bfc_f.astype(np.float32),
        "wp": Wp.astype(bf),
        "bp": bp.astype(np.float32),
        "ident": np.eye(128, dtype=bf),
        "maskt": np.triu(np.ones((128, 128))).astype(bf),
    }
    return x, common


_NC_CACHE = None


def get_module():
    global _NC_CACHE
    if _NC_CACHE is None:
        _NC_CACHE = build_module()
    return _NC_CACHE


def run_with_results(inputs, **run_kwargs):
    x, common = host_prepare(inputs)
    nc = get_module()
    in_maps = [dict(common, x=np.ascontiguousarray(x[b])) for b in range(B)]
    res = run_bass_kernel_spmd(nc, in_maps, core_ids=list(range(N_CORES)), **run_kwargs)
    out = np.stack([res.results[b]["out"] for b in range(B)]).astype(np.float32)
    return out, res


def kernel(**inputs):
    return run_with_results(inputs)[0]
